# revision 8
# baseline (speedup 1.0000x reference)
"""CornerNet Trainium2 kernel — rank-K separable expansion.

Math (reference):
  t     = kappa * tanh(sign_param) * (x - th)        # (B, R, D)
  s     = sigmoid(t); m = sigmoid(mask_logit)
  gated = 1 - m*(1-s)
  z     = prod_d gated                               # (B, R)
  y     = z @ head_w.T + head_b                      # (B,)

FAST PATH (mask_logit uniform, th == 0 — the actual model):
  ln z[b,r] = sum_d phi(a_rd * x_bd),  phi(u) = ln(1 - m + m*sigmoid(u))
The bivariate kernel phi(a*x) admits a rank-K separable expansion
  phi(a*x) ~= sum_k h_k(sp) * g_k(x)        (SVD of phi on a grid; h_k
                                             absorbs a = kappa*tanh(sp))
so the whole (B,R,D) elementwise pass collapses to K matmuls:
  ln z = sum_k G_k^T-contraction-F_k   with G_k = g_k(x), F_k = h_k(sp).

The arbitrary functions g_k / h_k / exp are evaluated in SINGLE ScalarE
passes by window-packing the `sigmoid_and_others` activation-table set
(selected because it holds sigmoid+tanh+erf+arctan -> ~1030 cubic spline
buckets in one load): each function gets a sub-interval of a table
function's stored side, addressed via the activation instruction's free
input affine (scale*x + bias); the buckets covering that window are refit
to the target function.  Inputs stay strictly on the stored side so
symmetry folding never fires, and all windows live in ONE table set (one
~2.7us ACT_TABLE_LOAD, no switches).

Sharding: data-parallel over batch: core c takes x columns [c*256,(c+1)*256)
and all R=512 rules; no cross-core reduction (host just concatenates).
Layout: D=256 on partitions (two 128-halves side by side on the free axis).
Per core / rep: ACT = K g-passes (128,512) + 1 exp pass (128,1024);
PE = 8K accumulating matmuls (N=256, f32r) + 4 head matmuls.

FALLBACK (non-uniform mask / th != 0 / missing scipy): the original
per-rule kernel (ScalarE bound) kept verbatim below.
"""

import hashlib
import json
import math
import os
import shutil
import tempfile

import numpy as np
from contextlib import ExitStack

import concourse.bass as bass
import concourse.bacc as bacc
import concourse.mybir as mybir
import concourse.tile as tile
from concourse.bass_utils import run_bass_kernel_spmd
from bass_rust import add_dep_helper

B, D, R = 2048, 256, 512
NCORES = 8
BC = B // NCORES            # 256 batch columns per core
K_RANK = 10
RC = R // NCORES            # legacy fallback: 64 rules per core
KBLK = 8
CH = 512
F32 = mybir.dt.float32
F32R = mybir.dt.float32r
AF = mybir.ActivationFunctionType
OP = mybir.AluOpType

_cache = {}

TABLE_VERSION = "rk4"

# ======================================================================
# Activation-table window toolkit (sigmoid_and_others set)
# ======================================================================

SET_NAME = "sigmoid_and_others"
# contiguous usable bucket index ranges (inclusive) per table function
BKT_RANGES = {
    "sigmoid": (139, 931),   # x0 -0.5625 .. -99.5625, uniform h=1/8 (desc)
    "tanh": (36, 123),       # x0 0.2656 .. 7.875
    "erf": (951, 1009),      # x0 0.2656 .. 3.66
    "arctan": (1046, 1141),  # x0 0.5156 .. 31.5 (log-spaced)
}
AF_OF = {"sigmoid": AF.Sigmoid, "tanh": AF.Tanh, "erf": AF.Erf,
         "arctan": AF.Arctan}
# uniform-resolution sub-segments usable as windows: (fn, u_lo, u_hi)
SEGMENTS = {
    "tanh_lo":  ("tanh",   0.266, 1.0, 1 / 32, 1 / 32),
    "tanh_mid": ("tanh",   1.0,   4.0, 1 / 16, 1 / 16),
    "erf_lo":   ("erf",    0.266, 1.0, 1 / 32, 1 / 32),
    "erf_mid":  ("erf",    1.0,   3.6, 1 / 16, 1 / 16),
    "atan_hi":  ("arctan", 16.0, 31.9, 1.0, 1.0),
}
SIG_LO, SIG_HI = -99.55, -0.50
SIG_H = 0.125


def _load_set(srcdir):
    d = json.load(open(os.path.join(srcdir, SET_NAME + ".json")))
    bkt = np.fromfile(
        os.path.join(srcdir, SET_NAME + "_bkt.bin"), dtype=np.float32
    ).reshape(d["bkt_entry_cnt"], 8)
    return d, bkt


def _region_table(bkt):
    out = {}
    for fn, (lo, hi) in BKT_RANGES.items():
        x0 = bkt[lo : hi + 1, 4].astype(np.float64)
        dx = np.abs(np.diff(x0))
        # fit over max of adjacent spacings: no extrapolation at octave edges
        w = np.empty_like(x0)
        w[0] = dx[0]
        w[1:-1] = np.maximum(dx[:-1], dx[1:])
        w[-1] = dx[-1]
        out[fn] = (lo, hi, x0, w)
    return out


def _fit_bucket(target, x0, w, npts=33):
    u = np.linspace(x0 - w / 2, x0 + w / 2, npts)
    y = target(u)
    A = np.vander(u - x0, 4, increasing=True)
    coef, *_ = np.linalg.lstsq(A, y, rcond=None)
    return coef


class _Window:
    """g(x) for x in [x_lo, x_hi] == F_fn(scale*x + bias), buckets refit."""

    def __init__(self, fn, u_lo, u_hi, x_lo, x_hi, g):
        self.fn = fn
        s = (u_hi - u_lo) / (x_hi - x_lo)
        b = u_lo - s * x_lo
        self.scale = float(np.float32(s))
        self.bias = float(np.float32(b))
        us = (self.scale * x_lo + self.bias, self.scale * x_hi + self.bias)
        self.u_lo, self.u_hi = min(us), max(us)
        self.g = g

    def apply(self, bkt, regions):
        lo, hi, x0, w = regions[self.fn]
        tgt = lambda u: self.g((np.asarray(u, np.float64) - self.bias) / self.scale)
        n = 0
        for i in range(len(x0)):
            if self.u_lo - 0.45 * w[i] <= x0[i] <= self.u_hi + 0.45 * w[i]:
                bkt[lo + i, 0:4] = _fit_bucket(tgt, x0[i], w[i]).astype(np.float32)
                n += 1
        return n


def _design_windows(mval, kappa, XM, SM, K):
    """SVD of phi(a*x) -> window plans for g_k (x-domain), h_k (sp-domain)
    and the final exp.  Pure numpy + scipy."""
    from scipy.interpolate import CubicSpline

    def phi(u):
        return np.logaddexp(np.log(1.0 - mval), u) - np.logaddexp(0.0, u)

    AMg = kappa * np.tanh(SM)
    ag = np.linspace(-AMg, AMg, 1001)
    xg = np.linspace(-XM, XM, 2001)
    U, S, Vt = np.linalg.svd(phi(np.outer(ag, xg)), full_matrices=False)
    gmax = np.abs(Vt[:K]).max(1)
    gs = [CubicSpline(xg, Vt[k] / gmax[k]) for k in range(K)]
    spg = np.linspace(-SM, SM, 6001)
    fs = [CubicSpline(ag, U[:, k] * S[k] * gmax[k]) for k in range(K)]
    hs = [CubicSpline(spg, fs[k](kappa * np.tanh(spg))) for k in range(K)]

    # sigmoid-band bucket counts per window (validated numerically: K=10
    # end-to-end 6.8e-4, K=12 needs the segment windows below to fit)
    ng = [24, 48, 48, 48, 48, 24, 24, 48, 24, 24, 32, 24]
    nh = [28, 64, 64, 64, 48, 28, 32, 48, 24, 24, 32, 24]
    seg_assign = {("g", 0): "tanh_mid", ("g", 5): "tanh_lo",
                  ("g", 6): "erf_mid", ("g", 8): "erf_lo"}

    cur = SIG_HI
    plans_g, plans_h = {}, {}

    def place_sig(n, dom_lo, dom_hi, g):
        nonlocal cur
        width = n * SIG_H
        u_hi = cur - SIG_H / 2
        u_lo = u_hi - (width - SIG_H)
        cur = cur - width - SIG_H
        assert cur > SIG_LO - SIG_H, "sigmoid band overflow"
        return _Window("sigmoid", u_lo, u_hi, dom_lo, dom_hi, g)

    def place_seg(name, dom_lo, dom_hi, g):
        fn, lo, hi, h_lo, h_hi = SEGMENTS[name]
        return _Window(fn, lo + 0.6 * h_lo, hi - 0.6 * h_hi, dom_lo, dom_hi, g)

    for k in range(K):
        if ("g", k) in seg_assign:
            plans_g[k] = place_seg(seg_assign[("g", k)], -XM, XM, gs[k])
        else:
            plans_g[k] = place_sig(ng[k], -XM, XM, gs[k])
    for k in range(K):
        if ("h", k) in seg_assign:
            plans_h[k] = place_seg(seg_assign[("h", k)], -SM, SM, hs[k])
        else:
            plans_h[k] = place_sig(nh[k], -SM, SM, hs[k])

    # exp on arctan's log-spaced [0.52, 15.9]: u = s*lz + b with s<0 puts the
    # top (important) lz octaves on the finest buckets; covers lz in
    # [-44, -10.8] with rel err <= ~2e-5 at the top
    plan_exp = _Window("arctan", 0.52, 15.9, -10.8, -44.0, np.exp)
    return plans_g, plans_h, plan_exp


def _get_design(mval, kappa, XM, SM, K):
    key = ("design", mval, kappa, XM, SM, K)
    if key not in _cache:
        _cache[key] = _design_windows(mval, kappa, XM, SM, K)
    return _cache[key]


def _gen_act_tables_rk(mval, kappa, XM, SM, K):
    """Build patched act-table dir; returns (act_info_json_path, tag)."""
    from neuronxcc.driver.Job import Job
    from neuronxcc.driver.jobs.support.FindActInfo import findActInfoFile

    src_json = findActInfoFile(Job.getPackageDir(), "gen3")
    srcdir = os.path.dirname(src_json)
    tag = hashlib.md5(
        (TABLE_VERSION + repr((float(mval), float(kappa), float(XM),
                               float(SM), int(K)))).encode()
    ).hexdigest()[:10]
    dstdir = os.path.join(tempfile.gettempdir(), f"cn_rk_{tag}")
    marker = os.path.join(dstdir, "act_info.json")
    if not os.path.isfile(marker):
        plans_g, plans_h, plan_exp = _get_design(mval, kappa, XM, SM, K)
        tmp = dstdir + ".tmp"
        shutil.rmtree(tmp, ignore_errors=True)
        os.makedirs(tmp)
        for f in os.listdir(srcdir):
            shutil.copyfile(os.path.join(srcdir, f), os.path.join(tmp, f))
        d, bkt = _load_set(tmp)
        regions = _region_table(bkt)
        for p in list(plans_g.values()) + list(plans_h.values()) + [plan_exp]:
            p.apply(bkt, regions)
        bkt.tofile(os.path.join(tmp, SET_NAME + "_bkt.bin"))
        shutil.rmtree(dstdir, ignore_errors=True)
        try:
            os.rename(tmp, dstdir)
        except OSError:
            if not os.path.isfile(marker):
                raise
    return marker, tag


# ======================================================================
# Fast kernel: rank-K expansion
# ======================================================================

def _build_rk(reps, tag, plans_g, plans_h, plan_exp, K, bf16_from=None,
              loop_n=0):
    NW = 2 * K + 1
    BF16 = mybir.dt.bfloat16
    def mm_dt(k):
        return BF16 if (bf16_from is not None and k >= bf16_from) else F32R
    nc = bacc.Bacc(None)
    xTc = nc.dram_tensor("xTc", [D, BC], F32, kind="ExternalInput")
    spT = nc.dram_tensor("spT", [D, R], F32, kind="ExternalInput")
    wrow = nc.dram_tensor("wrow", [128, R // 128], F32R, kind="ExternalInput")
    bvec = nc.dram_tensor(f"bv_{tag}", [128, NW], F32, kind="ExternalInput")
    y = nc.dram_tensor("y", [1, BC], F32, kind="ExternalOutput")
    NR = R // 128  # 4 rule chunks

    with tile.TileContext(nc) as tc, ExitStack() as ctx:
        const = ctx.enter_context(tc.tile_pool(name="const", bufs=1))
        gp = ctx.enter_context(tc.tile_pool(name="gp", bufs=4))
        zp = ctx.enter_context(tc.tile_pool(name="zp", bufs=2))
        psum = ctx.enter_context(
            tc.tile_pool(name="psum", bufs=1, space=bass.MemorySpace.PSUM)
        )

        # ---- input staging ----
        xt = const.tile([128, 2 * BC], F32, tag="xt")
        for h in range(2):
            nc.gpsimd.dma_start(xt[:, h * BC : (h + 1) * BC],
                                xTc[h * 128 : (h + 1) * 128, :])
        spt = const.tile([128, 2 * R], F32, tag="spt")
        for h in range(2):
            nc.sync.dma_start(spt[:, h * R : (h + 1) * R],
                              spT[h * 128 : (h + 1) * 128, :])
        wt = const.tile([128, NR], F32R, tag="wt")
        nc.sync.dma_start(wt[:], wrow[:])
        bt = const.tile([128, NW], F32, tag="bt")
        nc.sync.dma_start(bt[:], bvec[:])

        # ---- F_k = h_k(sp), once per execution ----
        fks = []
        for k in range(K):
            w = plans_h[k]
            fk = const.tile([128, 2 * R], mm_dt(k), tag=f"F{k}")
            nc.scalar.activation(fk[:], spt[:], AF_OF[w.fn],
                                 bias=bt[:, K + k : K + k + 1], scale=w.scale)
            fks.append(fk)

        # ---- main loop ----
        lzs = [psum.tile([128, NR * BC], F32, tag=f"lz{p}", name=f"lz{p}")
               for p in range(2)]
        yps = [psum.tile([1, BC], F32, tag=f"yp{p}", name=f"yp{p}")
               for p in range(2)]

        def emit_rep(par):
            lz = lzs[par]
            for k in range(K):
                w = plans_g[k]
                g = gp.tile([128, 2 * BC], mm_dt(k), tag="G", name="G")
                nc.scalar.activation(g[:], xt[:], AF_OF[w.fn],
                                     bias=bt[:, k : k + 1], scale=w.scale)
                for d in range(2):
                    for r in range(NR):
                        nc.tensor.matmul(
                            lz[:, r * BC : (r + 1) * BC],
                            fks[k][:, d * R + r * 128 : d * R + (r + 1) * 128],
                            g[:, d * BC : (d + 1) * BC],
                            start=(k == 0 and d == 0 and r % 2 == 0),
                            stop=(k == K - 1 and d == 1 and r % 2 == 1),
                        )
            z = zp.tile([128, NR * BC], F32R, tag="z", name="z")
            we = plan_exp
            nc.scalar.activation(z[:], lz[:], AF_OF[we.fn],
                                 bias=bt[:, 2 * K : 2 * K + 1], scale=we.scale)
            yp = yps[par]
            for r in range(NR):
                nc.tensor.matmul(yp[:], wt[:, r : r + 1],
                                 z[:, r * BC : (r + 1) * BC],
                                 start=(r == 0), stop=(r == NR - 1))
            ysb = zp.tile([1, BC], F32, tag="ysb", name="ysb")
            nc.vector.tensor_copy(ysb[:], yp[:])
            return ysb

        if loop_n:
            with tc.For_i(0, loop_n, 1):
                for rep in range(reps):
                    ysb = emit_rep(rep % 2)
        else:
            for rep in range(reps):
                ysb = emit_rep(rep % 2)
        nc.sync.dma_start(y[:], ysb[:])

    nc.compile()

    n_loads = sum(
        1
        for blk in nc.main_func.blocks
        for inst in blk.instructions
        if type(inst).__name__ == "InstLoadActFuncSet"
    )
    if n_loads != 1:
        raise RuntimeError(f"expected 1 act table load, got {n_loads}")
    return nc


def _get_nc_rk(reps, tag, plans, K, loop_n=0):
    bf16_from = _variant_bf16_from()
    key = ("rk", reps, tag, K, bf16_from, loop_n)
    if key not in _cache:
        _cache[key] = _build_rk(reps, tag, *plans, K, bf16_from=bf16_from,
                                loop_n=loop_n)
    return _cache[key]


def _variant_bf16_from():
    v = os.environ.get("CN_BF16_FROM", "")
    if v == "":
        return None
    return int(v)


def _make_in_maps_rk(inputs, tag, plans_g, plans_h, plan_exp, K):
    x = np.ascontiguousarray(np.asarray(inputs["x"], dtype=np.float32))
    sp = np.asarray(inputs["sign_param"], dtype=np.float32)
    hw = np.asarray(inputs["head_w"], dtype=np.float32).reshape(-1)

    xT = np.ascontiguousarray(x.T)                  # (D, B)
    spT = np.ascontiguousarray(sp.T)                # (D, R)
    wrow = np.ascontiguousarray(hw.reshape(R // 128, 128).T)  # (128, R/128)
    biases = ([plans_g[k].bias for k in range(K)]
              + [plans_h[k].bias for k in range(K)] + [plan_exp.bias])
    bvec = np.tile(np.asarray(biases, np.float32), (128, 1))

    in_maps = []
    for c in range(NCORES):
        in_maps.append({
            "xTc": np.ascontiguousarray(xT[:, c * BC : (c + 1) * BC]),
            "spT": spT,
            "wrow": wrow,
            f"bv_{tag}": bvec,
        })
    return in_maps


def _fast_path_params(inputs):
    """Return (m, kappa, XM, SM) if the fast path applies, else None."""
    try:
        import scipy.interpolate  # noqa: F401
    except Exception:
        return None
    th = np.asarray(inputs["th"], dtype=np.float64)
    if th.shape != (R, D) or np.any(th != 0.0):
        return None
    mk = np.asarray(inputs["mask_logit"], dtype=np.float64)
    v = mk.reshape(-1)[0]
    if mk.shape != (R, D) or not np.all(mk == v):
        return None
    x = np.asarray(inputs["x"], dtype=np.float64)
    sp = np.asarray(inputs["sign_param"], dtype=np.float64)
    if x.shape != (B, D) or sp.shape != (R, D):
        return None
    mval = 1.0 / (1.0 + np.exp(-v))
    kappa = math.exp(float(np.asarray(inputs["log_kappa"]).reshape(-1)[0]))
    XM = float(np.float32(np.abs(x).max() * 1.06))
    SM = float(np.float32(np.abs(sp).max() * 1.06))
    if not (0.05 < mval < 0.95) or not (0.05 < kappa < 100.0):
        return None
    if XM > 30.0 or SM > 0.9 or kappa * np.tanh(SM) * XM > 60.0:
        return None
    return float(mval), float(kappa), XM, SM


def _run_rk(inputs, params, reps=1, **spmd_kwargs):
    mval, kappa, XM, SM = params
    K = K_RANK
    json_path, tag = _gen_act_tables_rk(mval, kappa, XM, SM, K)
    os.environ["BASS_ACT_ROOT_JSON_PATH"] = json_path
    plans = _get_design(mval, kappa, XM, SM, K)
    nc = _get_nc_rk(reps, tag, plans, K)
    in_maps = _make_in_maps_rk(inputs, tag, *plans, K)
    res = run_bass_kernel_spmd(nc, in_maps, core_ids=list(range(NCORES)),
                               **spmd_kwargs)
    hb = np.asarray(inputs["head_b"], dtype=np.float32).reshape(-1)[0]
    y = np.concatenate([r["y"][0] for r in res.results]) + hb
    return y.astype(np.float32), res


# ======================================================================
# Legacy fallback kernel (per-rule ScalarE path) — kept verbatim
# ======================================================================

def _build(reps=1):
    nc = bacc.Bacc(None)
    xT = nc.dram_tensor("xT", [D, B], F32, kind="ExternalInput")
    thT = nc.dram_tensor("thT", [D, RC], F32, kind="ExternalInput")
    sgT = nc.dram_tensor("sgT", [D, RC], F32, kind="ExternalInput")
    mkT = nc.dram_tensor("mkT", [D, RC], F32, kind="ExternalInput")
    lkb = nc.dram_tensor("lkb", [128, 1], F32, kind="ExternalInput")
    wcol = nc.dram_tensor("wcol", [RC, 1], F32, kind="ExternalInput")
    selp = nc.dram_tensor("selp", [128, 2 * RC], F32R, kind="ExternalInput")
    y = nc.dram_tensor("y", [1, B], F32, kind="ExternalOutput")

    with tile.TileContext(nc) as tc, ExitStack() as ctx:
        const = ctx.enter_context(tc.tile_pool(name="const", bufs=1))
        sp = ctx.enter_context(tc.tile_pool(name="sp", bufs=2))
        gp_ = ctx.enter_context(tc.tile_pool(name="gp_", bufs=2))
        gpp = ctx.enter_context(tc.tile_pool(name="gpp", bufs=KBLK + 1))
        lp = ctx.enter_context(tc.tile_pool(name="lp", bufs=2))
        psum = ctx.enter_context(
            tc.tile_pool(name="psum", bufs=1, space=bass.MemorySpace.PSUM)
        )

        xt = []
        for h in range(2):
            t_ = const.tile([128, B], F32, tag=f"xt{h}")
            nc.gpsimd.dma_start(t_[:], xT[h * 128 : (h + 1) * 128, :])
            xt.append(t_)

        tht, sgt, mkt = [], [], []
        for name, dram, lst in (("th", thT, tht), ("sg", sgT, sgt), ("mk", mkT, mkt)):
            for h in range(2):
                t_ = const.tile([128, RC], F32, tag=f"{name}{h}")
                nc.gpsimd.dma_start(t_[:], dram[h * 128 : (h + 1) * 128, :])
                lst.append(t_)

        lkt = const.tile([128, 1], F32, tag="lkt")
        nc.gpsimd.dma_start(lkt[:], lkb[:])
        selpt = const.tile([128, 2 * RC], F32R, tag="selpt")
        nc.gpsimd.dma_start(selpt[:], selp[:])
        wct = const.tile([RC, 1], F32, tag="wct")
        nc.gpsimd.dma_start(wct[:], wcol[:])

        kap = const.tile([128, 1], F32, tag="kap")
        nc.scalar.activation(kap[:], lkt[:], AF.Exp)
        nkap = const.tile([128, 1], F32, tag="nkap")
        nc.vector.tensor_scalar(nkap[:], kap[:], -1.0, None, OP.mult)

        aa, nb2, mm_, cc_ = [], [], [], []
        for h in range(2):
            tnh = const.tile([128, RC], F32, tag=f"tnh{h}")
            nc.scalar.activation(tnh[:], sgt[h][:], AF.Tanh)
            a_h = const.tile([128, RC], F32, tag=f"a{h}")
            nc.vector.tensor_scalar(a_h[:], tnh[:], kap[:], None, OP.mult)
            na_h = const.tile([128, RC], F32, tag=f"na{h}")
            nc.vector.tensor_scalar(na_h[:], tnh[:], nkap[:], None, OP.mult)
            nb2_h = const.tile([128, RC], F32, tag=f"nb2{h}")
            nc.vector.tensor_mul(nb2_h[:], na_h[:], tht[h][:])
            aa.append(a_h)
            nb2.append(nb2_h)
            m_h = const.tile([128, RC], F32, tag=f"m{h}")
            nc.scalar.activation(m_h[:], mkt[h][:], AF.Sigmoid)
            c_h = const.tile([128, RC], F32, tag=f"c{h}")
            nc.scalar.activation(c_h[:], mkt[h][:], AF.Sigmoid, scale=-1.0)
            mm_.append(m_h)
            cc_.append(c_h)

        lz = psum.tile([RC, B], F32, tag="lz")
        last_ln = None
        for rep in range(reps):
            for blk in range(RC // KBLK):
                gps = []
                sig_insts = []
                for k in range(KBLK):
                    r = blk * KBLK + k
                    s = sp.tile([128, 2 * B], F32, tag="s")
                    for h in range(2):
                        si = nc.scalar.activation(
                            s[:, h * B : (h + 1) * B],
                            xt[h][:],
                            AF.Sigmoid,
                            bias=nb2[h][:, r : r + 1],
                            scale=aa[h][:, r : r + 1],
                        )
                        if last_ln is not None:
                            add_dep_helper(si.ins, last_ln.ins, False,
                                           "act-table phase blocking")
                        sig_insts.append(si)
                    g = gp_.tile([128, 2 * B], F32, tag="g")
                    for h in range(2):
                        nc.vector.tensor_scalar(
                            g[:, h * B : (h + 1) * B],
                            s[:, h * B : (h + 1) * B],
                            mm_[h][:, r : r + 1],
                            cc_[h][:, r : r + 1],
                            OP.mult,
                            OP.add,
                        )
                    gpt = gpp.tile([128, B], F32, tag="gpt")
                    nc.vector.tensor_mul(gpt[:], g[:, 0:B], g[:, B : 2 * B])
                    gps.append(gpt)
                for k in range(KBLK):
                    r = blk * KBLK + k
                    L = lp.tile([128, B], F32R, tag="L")
                    ln_i = nc.scalar.activation(L[:], gps[k][:], AF.Ln)
                    add_dep_helper(ln_i.ins, sig_insts[-1].ins, False,
                                   "act-table phase blocking")
                    last_ln = ln_i
                    lhsp = selpt[:, RC - r : 2 * RC - r]
                    for c in range(B // CH):
                        nc.tensor.matmul(
                            lz[:, c * CH : (c + 1) * CH],
                            lhsp,
                            L[:, c * CH : (c + 1) * CH],
                            start=(r == 0 and rep == 0),
                            stop=(r == RC - 1 and rep == reps - 1),
                        )

        z_sb = const.tile([RC, B], F32, tag="z")
        nc.scalar.activation(z_sb[:], lz[:], AF.Exp)
        yp = psum.tile([1, B], F32, tag="yp")
        for c in range(B // CH):
            nc.tensor.matmul(
                yp[:, c * CH : (c + 1) * CH],
                wct[:],
                z_sb[:, c * CH : (c + 1) * CH],
                start=True,
                stop=True,
            )
        y_sb = const.tile([1, B], F32, tag="ysb")
        nc.vector.tensor_copy(y_sb[:], yp[:])
        nc.sync.dma_start(y[:], y_sb[:])

    nc.compile()
    return nc


def _get_nc(reps=1):
    key = ("nc", reps)
    if key not in _cache:
        _cache[key] = _build(reps)
    return _cache[key]


def _make_in_maps(inputs):
    x = np.ascontiguousarray(inputs["x"], dtype=np.float32)
    th = np.asarray(inputs["th"], dtype=np.float32)
    sg = np.asarray(inputs["sign_param"], dtype=np.float32)
    mk = np.asarray(inputs["mask_logit"], dtype=np.float32)
    lk = float(np.asarray(inputs["log_kappa"], dtype=np.float32).reshape(-1)[0])
    hw = np.asarray(inputs["head_w"], dtype=np.float32)

    xT = np.ascontiguousarray(x.T)
    lkb = np.full((128, 1), lk, dtype=np.float32)
    selp = np.zeros((128, 2 * RC), dtype=np.float32)
    selp[:, RC] = 1.0

    in_maps = []
    for c in range(NCORES):
        sl = slice(c * RC, (c + 1) * RC)
        in_maps.append(
            {
                "xT": xT,
                "thT": np.ascontiguousarray(th[sl].T),
                "sgT": np.ascontiguousarray(sg[sl].T),
                "mkT": np.ascontiguousarray(mk[sl].T),
                "lkb": lkb,
                "wcol": np.ascontiguousarray(hw.reshape(-1)[sl].reshape(RC, 1)),
                "selp": selp,
            }
        )
    return in_maps


def _run_legacy(inputs, reps=1, **spmd_kwargs):
    os.environ.pop("BASS_ACT_ROOT_JSON_PATH", None)
    nc = _get_nc(reps)
    in_maps = _make_in_maps(inputs)
    res = run_bass_kernel_spmd(nc, in_maps, core_ids=list(range(NCORES)),
                               **spmd_kwargs)
    hb = np.asarray(inputs["head_b"], dtype=np.float32).reshape(-1)[0]
    y = np.sum([r["y"][0] for r in res.results], axis=0, dtype=np.float32) + hb
    return y.astype(np.float32), res


def _run(inputs, reps=1, **spmd_kwargs):
    params = _fast_path_params(inputs)
    if params is not None:
        return _run_rk(inputs, params, reps=reps, **spmd_kwargs)
    return _run_legacy(inputs, reps=reps, **spmd_kwargs)


def kernel(**inputs) -> np.ndarray:
    y, _ = _run(inputs)
    return y


# revision 12
# speedup vs baseline: 1.3015x; 1.3015x over previous
"""CornerNet Trainium2 kernel — rank-K separable expansion.

Math (reference):
  t     = kappa * tanh(sign_param) * (x - th)        # (B, R, D)
  s     = sigmoid(t); m = sigmoid(mask_logit)
  gated = 1 - m*(1-s)
  z     = prod_d gated                               # (B, R)
  y     = z @ head_w.T + head_b                      # (B,)

FAST PATH (mask_logit uniform, th == 0 — the actual model):
  ln z[b,r] = sum_d phi(a_rd * x_bd),  phi(u) = ln(1 - m + m*sigmoid(u))
The bivariate kernel phi(a*x) admits a rank-K separable expansion
  phi(a*x) ~= sum_k h_k(sp) * g_k(x)        (SVD of phi on a grid; h_k
                                             absorbs a = kappa*tanh(sp))
so the whole (B,R,D) elementwise pass collapses to K matmuls:
  ln z = sum_k G_k^T-contraction-F_k   with G_k = g_k(x), F_k = h_k(sp).

The arbitrary functions g_k / h_k / exp are evaluated in SINGLE ScalarE
passes by window-packing the `sigmoid_and_others` activation-table set
(selected because it holds sigmoid+tanh+erf+arctan -> ~1030 cubic spline
buckets in one load): each function gets a sub-interval of a table
function's stored side, addressed via the activation instruction's free
input affine (scale*x + bias); the buckets covering that window are refit
to the target function.  Inputs stay strictly on the stored side so
symmetry folding never fires, and all windows live in ONE table set (one
~2.7us ACT_TABLE_LOAD, no switches).

Sharding: data-parallel over batch: core c takes x columns [c*256,(c+1)*256)
and all R=512 rules; no cross-core reduction (host just concatenates).
Layout: D=256 on partitions (two 128-halves side by side on the free axis).
Per core / rep: ACT = K g-passes (128,512) + 1 exp pass (128,1024);
PE = 8K accumulating matmuls (N=256; f32r for k<2, bf16 for k>=2 whose
coefficients are small enough that bf16 noise is negligible) + 4 head
matmuls.  The exp+head tail of rep i is emitted inside rep i+1's k-loop so
the strict-FIFO ACT queue never starves PE at the rep boundary.  PSUM
gotcha: start=True clears has_written for the whole BANK, so exactly one
start/stop per bank per accumulation group.  Defaults K=8 (rel err ~1.3e-3
vs the 2e-2 gate); CN_K / CN_BF16_FROM env override for experiments.
Measured steady-state ~8.6us/rep vs 343us/164us for the per-rule baseline.

FALLBACK (non-uniform mask / th != 0 / missing scipy): the original
per-rule kernel (ScalarE bound) kept verbatim below.
"""

import hashlib
import json
import math
import os
import shutil
import tempfile

import numpy as np
from contextlib import ExitStack

import concourse.bass as bass
import concourse.bacc as bacc
import concourse.mybir as mybir
import concourse.tile as tile
from concourse.bass_utils import run_bass_kernel_spmd
from bass_rust import add_dep_helper

B, D, R = 2048, 256, 512
NCORES = 8
BC = B // NCORES            # 256 batch columns per core
K_RANK = int(os.environ.get("CN_K", "8"))
RC = R // NCORES            # legacy fallback: 64 rules per core
KBLK = 8
CH = 512
F32 = mybir.dt.float32
F32R = mybir.dt.float32r
AF = mybir.ActivationFunctionType
OP = mybir.AluOpType

_cache = {}

TABLE_VERSION = "rk4"

# ======================================================================
# Activation-table window toolkit (sigmoid_and_others set)
# ======================================================================

SET_NAME = "sigmoid_and_others"
# contiguous usable bucket index ranges (inclusive) per table function
BKT_RANGES = {
    "sigmoid": (139, 931),   # x0 -0.5625 .. -99.5625, uniform h=1/8 (desc)
    "tanh": (36, 123),       # x0 0.2656 .. 7.875
    "erf": (951, 1009),      # x0 0.2656 .. 3.66
    "arctan": (1046, 1141),  # x0 0.5156 .. 31.5 (log-spaced)
}
AF_OF = {"sigmoid": AF.Sigmoid, "tanh": AF.Tanh, "erf": AF.Erf,
         "arctan": AF.Arctan}
# uniform-resolution sub-segments usable as windows: (fn, u_lo, u_hi)
SEGMENTS = {
    "tanh_lo":  ("tanh",   0.266, 1.0, 1 / 32, 1 / 32),
    "tanh_mid": ("tanh",   1.0,   4.0, 1 / 16, 1 / 16),
    "erf_lo":   ("erf",    0.266, 1.0, 1 / 32, 1 / 32),
    "erf_mid":  ("erf",    1.0,   3.6, 1 / 16, 1 / 16),
    "atan_hi":  ("arctan", 16.0, 31.9, 1.0, 1.0),
}
SIG_LO, SIG_HI = -99.55, -0.50
SIG_H = 0.125


def _load_set(srcdir):
    d = json.load(open(os.path.join(srcdir, SET_NAME + ".json")))
    bkt = np.fromfile(
        os.path.join(srcdir, SET_NAME + "_bkt.bin"), dtype=np.float32
    ).reshape(d["bkt_entry_cnt"], 8)
    return d, bkt


def _region_table(bkt):
    out = {}
    for fn, (lo, hi) in BKT_RANGES.items():
        x0 = bkt[lo : hi + 1, 4].astype(np.float64)
        dx = np.abs(np.diff(x0))
        # fit over max of adjacent spacings: no extrapolation at octave edges
        w = np.empty_like(x0)
        w[0] = dx[0]
        w[1:-1] = np.maximum(dx[:-1], dx[1:])
        w[-1] = dx[-1]
        out[fn] = (lo, hi, x0, w)
    return out


def _fit_bucket(target, x0, w, npts=33):
    u = np.linspace(x0 - w / 2, x0 + w / 2, npts)
    y = target(u)
    A = np.vander(u - x0, 4, increasing=True)
    coef, *_ = np.linalg.lstsq(A, y, rcond=None)
    return coef


class _Window:
    """g(x) for x in [x_lo, x_hi] == F_fn(scale*x + bias), buckets refit."""

    def __init__(self, fn, u_lo, u_hi, x_lo, x_hi, g):
        self.fn = fn
        s = (u_hi - u_lo) / (x_hi - x_lo)
        b = u_lo - s * x_lo
        self.scale = float(np.float32(s))
        self.bias = float(np.float32(b))
        us = (self.scale * x_lo + self.bias, self.scale * x_hi + self.bias)
        self.u_lo, self.u_hi = min(us), max(us)
        self.g = g

    def apply(self, bkt, regions):
        lo, hi, x0, w = regions[self.fn]
        tgt = lambda u: self.g((np.asarray(u, np.float64) - self.bias) / self.scale)
        n = 0
        for i in range(len(x0)):
            if self.u_lo - 0.45 * w[i] <= x0[i] <= self.u_hi + 0.45 * w[i]:
                bkt[lo + i, 0:4] = _fit_bucket(tgt, x0[i], w[i]).astype(np.float32)
                n += 1
        return n


def _design_windows(mval, kappa, XM, SM, K):
    """SVD of phi(a*x) -> window plans for g_k (x-domain), h_k (sp-domain)
    and the final exp.  Pure numpy + scipy."""
    from scipy.interpolate import CubicSpline

    def phi(u):
        return np.logaddexp(np.log(1.0 - mval), u) - np.logaddexp(0.0, u)

    AMg = kappa * np.tanh(SM)
    ag = np.linspace(-AMg, AMg, 1001)
    xg = np.linspace(-XM, XM, 2001)
    U, S, Vt = np.linalg.svd(phi(np.outer(ag, xg)), full_matrices=False)
    gmax = np.abs(Vt[:K]).max(1)
    gs = [CubicSpline(xg, Vt[k] / gmax[k]) for k in range(K)]
    spg = np.linspace(-SM, SM, 6001)
    fs = [CubicSpline(ag, U[:, k] * S[k] * gmax[k]) for k in range(K)]
    hs = [CubicSpline(spg, fs[k](kappa * np.tanh(spg))) for k in range(K)]

    # sigmoid-band bucket counts per window (validated numerically: K=10
    # end-to-end 6.8e-4, K=12 needs the segment windows below to fit)
    ng = [24, 48, 48, 48, 48, 24, 24, 48, 24, 24, 32, 24]
    nh = [28, 64, 64, 64, 48, 28, 32, 48, 24, 24, 32, 24]
    seg_assign = {("g", 0): "tanh_mid", ("g", 5): "tanh_lo",
                  ("g", 6): "erf_mid", ("g", 8): "erf_lo"}

    cur = SIG_HI
    plans_g, plans_h = {}, {}

    def place_sig(n, dom_lo, dom_hi, g):
        nonlocal cur
        width = n * SIG_H
        u_hi = cur - SIG_H / 2
        u_lo = u_hi - (width - SIG_H)
        cur = cur - width - SIG_H
        assert cur > SIG_LO - SIG_H, "sigmoid band overflow"
        return _Window("sigmoid", u_lo, u_hi, dom_lo, dom_hi, g)

    def place_seg(name, dom_lo, dom_hi, g):
        fn, lo, hi, h_lo, h_hi = SEGMENTS[name]
        return _Window(fn, lo + 0.6 * h_lo, hi - 0.6 * h_hi, dom_lo, dom_hi, g)

    for k in range(K):
        if ("g", k) in seg_assign:
            plans_g[k] = place_seg(seg_assign[("g", k)], -XM, XM, gs[k])
        else:
            plans_g[k] = place_sig(ng[k], -XM, XM, gs[k])
    for k in range(K):
        if ("h", k) in seg_assign:
            plans_h[k] = place_seg(seg_assign[("h", k)], -SM, SM, hs[k])
        else:
            plans_h[k] = place_sig(nh[k], -SM, SM, hs[k])

    # exp on arctan's log-spaced [0.52, 15.9]: u = s*lz + b with s<0 puts the
    # top (important) lz octaves on the finest buckets; covers lz in
    # [-44, -10.8] with rel err <= ~2e-5 at the top
    plan_exp = _Window("arctan", 0.52, 15.9, -10.8, -44.0, np.exp)
    return plans_g, plans_h, plan_exp


def _get_design(mval, kappa, XM, SM, K):
    key = ("design", mval, kappa, XM, SM, K)
    if key not in _cache:
        _cache[key] = _design_windows(mval, kappa, XM, SM, K)
    return _cache[key]


def _gen_act_tables_rk(mval, kappa, XM, SM, K):
    """Build patched act-table dir; returns (act_info_json_path, tag)."""
    from neuronxcc.driver.Job import Job
    from neuronxcc.driver.jobs.support.FindActInfo import findActInfoFile

    src_json = findActInfoFile(Job.getPackageDir(), "gen3")
    srcdir = os.path.dirname(src_json)
    tag = hashlib.md5(
        (TABLE_VERSION + repr((float(mval), float(kappa), float(XM),
                               float(SM), int(K)))).encode()
    ).hexdigest()[:10]
    dstdir = os.path.join(tempfile.gettempdir(), f"cn_rk_{tag}")
    marker = os.path.join(dstdir, "act_info.json")
    if not os.path.isfile(marker):
        plans_g, plans_h, plan_exp = _get_design(mval, kappa, XM, SM, K)
        tmp = dstdir + ".tmp"
        shutil.rmtree(tmp, ignore_errors=True)
        os.makedirs(tmp)
        for f in os.listdir(srcdir):
            shutil.copyfile(os.path.join(srcdir, f), os.path.join(tmp, f))
        d, bkt = _load_set(tmp)
        regions = _region_table(bkt)
        for p in list(plans_g.values()) + list(plans_h.values()) + [plan_exp]:
            p.apply(bkt, regions)
        bkt.tofile(os.path.join(tmp, SET_NAME + "_bkt.bin"))
        shutil.rmtree(dstdir, ignore_errors=True)
        try:
            os.rename(tmp, dstdir)
        except OSError:
            if not os.path.isfile(marker):
                raise
    return marker, tag


# ======================================================================
# Fast kernel: rank-K expansion
# ======================================================================

def _build_rk(reps, tag, plans_g, plans_h, plan_exp, K, bf16_from=None,
              loop_n=0):
    NW = 2 * K + 1
    BF16 = mybir.dt.bfloat16
    def mm_dt(k):
        return BF16 if (bf16_from is not None and k >= bf16_from) else F32R
    nc = bacc.Bacc(None)
    xTc = nc.dram_tensor("xTc", [D, BC], F32, kind="ExternalInput")
    spT = nc.dram_tensor("spT", [D, R], F32, kind="ExternalInput")
    wrow = nc.dram_tensor("wrow", [128, R // 128], F32R, kind="ExternalInput")
    bvec = nc.dram_tensor(f"bv_{tag}", [128, NW], F32, kind="ExternalInput")
    y = nc.dram_tensor("y", [1, BC], F32, kind="ExternalOutput")
    NR = R // 128  # 4 rule chunks

    with tile.TileContext(nc) as tc, ExitStack() as ctx:
        const = ctx.enter_context(tc.tile_pool(name="const", bufs=1))
        gp = ctx.enter_context(tc.tile_pool(name="gp", bufs=4))
        zp = ctx.enter_context(tc.tile_pool(name="zp", bufs=2))
        psum = ctx.enter_context(
            tc.tile_pool(name="psum", bufs=1, space=bass.MemorySpace.PSUM)
        )

        # ---- input staging ----
        xt = const.tile([128, 2 * BC], F32, tag="xt")
        for h in range(2):
            nc.gpsimd.dma_start(xt[:, h * BC : (h + 1) * BC],
                                xTc[h * 128 : (h + 1) * 128, :])
        spt = const.tile([128, 2 * R], F32, tag="spt")
        for h in range(2):
            nc.sync.dma_start(spt[:, h * R : (h + 1) * R],
                              spT[h * 128 : (h + 1) * 128, :])
        wt = const.tile([128, NR], F32R, tag="wt")
        nc.sync.dma_start(wt[:], wrow[:])
        bt = const.tile([128, NW], F32, tag="bt")
        nc.sync.dma_start(bt[:], bvec[:])

        # ---- F_k = h_k(sp), once per execution ----
        fks = []
        for k in range(K):
            w = plans_h[k]
            fk = const.tile([128, 2 * R], mm_dt(k), tag=f"F{k}")
            nc.scalar.activation(fk[:], spt[:], AF_OF[w.fn],
                                 bias=bt[:, K + k : K + k + 1], scale=w.scale)
            fks.append(fk)

        # ---- main loop ----
        lzs = [psum.tile([128, NR * BC], F32, tag=f"lz{p}", name=f"lz{p}")
               for p in range(2)]
        yps = [psum.tile([1, BC], F32, tag=f"yp{p}", name=f"yp{p}")
               for p in range(2)]

        def emit_tail(par):
            # exp + head + copy for the accumulation that finished in lzs[par]
            lz = lzs[par]
            z = zp.tile([128, NR * BC], F32R, tag="z", name="z")
            we = plan_exp
            nc.scalar.activation(z[:], lz[:], AF_OF[we.fn],
                                 bias=bt[:, 2 * K : 2 * K + 1], scale=we.scale)
            yp = yps[par]
            for r in range(NR):
                nc.tensor.matmul(yp[:], wt[:, r : r + 1],
                                 z[:, r * BC : (r + 1) * BC],
                                 start=(r == 0), stop=(r == NR - 1))
            ysb = zp.tile([1, BC], F32, tag="ysb", name="ysb")
            nc.vector.tensor_copy(ysb[:], yp[:])
            return ysb

        def emit_body(par, pending):
            # the k-loop for this rep; the PREVIOUS rep's exp/head tail is
            # emitted mid-loop so ACT never starves PE at the boundary
            lz = lzs[par]
            ysb = None
            for k in range(K):
                w = plans_g[k]
                g = gp.tile([128, 2 * BC], mm_dt(k), tag="G", name="G")
                nc.scalar.activation(g[:], xt[:], AF_OF[w.fn],
                                     bias=bt[:, k : k + 1], scale=w.scale)
                for d in range(2):
                    for r in range(NR):
                        nc.tensor.matmul(
                            lz[:, r * BC : (r + 1) * BC],
                            fks[k][:, d * R + r * 128 : d * R + (r + 1) * 128],
                            g[:, d * BC : (d + 1) * BC],
                            start=(k == 0 and d == 0 and r % 2 == 0),
                            stop=(k == K - 1 and d == 1 and r % 2 == 1),
                        )
                if k == 1 and pending is not None:
                    ysb = emit_tail(pending)
            return ysb

        def emit_reps(n):
            pending = None
            ysb = None
            for rep in range(n):
                t = emit_body(rep % 2, pending)
                ysb = t if t is not None else ysb
                pending = rep % 2
            ysb2 = emit_tail(pending)
            return ysb2

        if loop_n:
            with tc.For_i(0, loop_n, 1):
                ysb = emit_reps(reps)
        else:
            ysb = emit_reps(reps)
        nc.sync.dma_start(y[:], ysb[:])

    nc.compile()

    n_loads = sum(
        1
        for blk in nc.main_func.blocks
        for inst in blk.instructions
        if type(inst).__name__ == "InstLoadActFuncSet"
    )
    if n_loads != 1:
        raise RuntimeError(f"expected 1 act table load, got {n_loads}")
    return nc


def _get_nc_rk(reps, tag, plans, K, loop_n=0):
    bf16_from = _variant_bf16_from()
    key = ("rk", reps, tag, K, bf16_from, loop_n)
    if key not in _cache:
        _cache[key] = _build_rk(reps, tag, *plans, K, bf16_from=bf16_from,
                                loop_n=loop_n)
    return _cache[key]


def _variant_bf16_from():
    v = os.environ.get("CN_BF16_FROM", "2")
    if v in ("", "none"):
        return None
    return int(v)


def _make_in_maps_rk(inputs, tag, plans_g, plans_h, plan_exp, K):
    x = np.ascontiguousarray(np.asarray(inputs["x"], dtype=np.float32))
    sp = np.asarray(inputs["sign_param"], dtype=np.float32)
    hw = np.asarray(inputs["head_w"], dtype=np.float32).reshape(-1)

    xT = np.ascontiguousarray(x.T)                  # (D, B)
    spT = np.ascontiguousarray(sp.T)                # (D, R)
    wrow = np.ascontiguousarray(hw.reshape(R // 128, 128).T)  # (128, R/128)
    biases = ([plans_g[k].bias for k in range(K)]
              + [plans_h[k].bias for k in range(K)] + [plan_exp.bias])
    bvec = np.tile(np.asarray(biases, np.float32), (128, 1))

    in_maps = []
    for c in range(NCORES):
        in_maps.append({
            "xTc": np.ascontiguousarray(xT[:, c * BC : (c + 1) * BC]),
            "spT": spT,
            "wrow": wrow,
            f"bv_{tag}": bvec,
        })
    return in_maps


def _fast_path_params(inputs):
    """Return (m, kappa, XM, SM) if the fast path applies, else None."""
    try:
        import scipy.interpolate  # noqa: F401
    except Exception:
        return None
    th = np.asarray(inputs["th"], dtype=np.float64)
    if th.shape != (R, D) or np.any(th != 0.0):
        return None
    mk = np.asarray(inputs["mask_logit"], dtype=np.float64)
    v = mk.reshape(-1)[0]
    if mk.shape != (R, D) or not np.all(mk == v):
        return None
    x = np.asarray(inputs["x"], dtype=np.float64)
    sp = np.asarray(inputs["sign_param"], dtype=np.float64)
    if x.shape != (B, D) or sp.shape != (R, D):
        return None
    mval = 1.0 / (1.0 + np.exp(-v))
    kappa = math.exp(float(np.asarray(inputs["log_kappa"]).reshape(-1)[0]))
    XM = float(np.float32(np.abs(x).max() * 1.06))
    SM = float(np.float32(np.abs(sp).max() * 1.06))
    if not (0.05 < mval < 0.95) or not (0.05 < kappa < 100.0):
        return None
    if XM > 30.0 or SM > 0.9 or kappa * np.tanh(SM) * XM > 60.0:
        return None
    return float(mval), float(kappa), XM, SM


def _run_rk(inputs, params, reps=1, **spmd_kwargs):
    mval, kappa, XM, SM = params
    K = K_RANK
    json_path, tag = _gen_act_tables_rk(mval, kappa, XM, SM, K)
    os.environ["BASS_ACT_ROOT_JSON_PATH"] = json_path
    plans = _get_design(mval, kappa, XM, SM, K)
    nc = _get_nc_rk(reps, tag, plans, K)
    in_maps = _make_in_maps_rk(inputs, tag, *plans, K)
    res = run_bass_kernel_spmd(nc, in_maps, core_ids=list(range(NCORES)),
                               **spmd_kwargs)
    hb = np.asarray(inputs["head_b"], dtype=np.float32).reshape(-1)[0]
    y = np.concatenate([r["y"][0] for r in res.results]) + hb
    return y.astype(np.float32), res


# ======================================================================
# Legacy fallback kernel (per-rule ScalarE path) — kept verbatim
# ======================================================================

def _build(reps=1):
    nc = bacc.Bacc(None)
    xT = nc.dram_tensor("xT", [D, B], F32, kind="ExternalInput")
    thT = nc.dram_tensor("thT", [D, RC], F32, kind="ExternalInput")
    sgT = nc.dram_tensor("sgT", [D, RC], F32, kind="ExternalInput")
    mkT = nc.dram_tensor("mkT", [D, RC], F32, kind="ExternalInput")
    lkb = nc.dram_tensor("lkb", [128, 1], F32, kind="ExternalInput")
    wcol = nc.dram_tensor("wcol", [RC, 1], F32, kind="ExternalInput")
    selp = nc.dram_tensor("selp", [128, 2 * RC], F32R, kind="ExternalInput")
    y = nc.dram_tensor("y", [1, B], F32, kind="ExternalOutput")

    with tile.TileContext(nc) as tc, ExitStack() as ctx:
        const = ctx.enter_context(tc.tile_pool(name="const", bufs=1))
        sp = ctx.enter_context(tc.tile_pool(name="sp", bufs=2))
        gp_ = ctx.enter_context(tc.tile_pool(name="gp_", bufs=2))
        gpp = ctx.enter_context(tc.tile_pool(name="gpp", bufs=KBLK + 1))
        lp = ctx.enter_context(tc.tile_pool(name="lp", bufs=2))
        psum = ctx.enter_context(
            tc.tile_pool(name="psum", bufs=1, space=bass.MemorySpace.PSUM)
        )

        xt = []
        for h in range(2):
            t_ = const.tile([128, B], F32, tag=f"xt{h}")
            nc.gpsimd.dma_start(t_[:], xT[h * 128 : (h + 1) * 128, :])
            xt.append(t_)

        tht, sgt, mkt = [], [], []
        for name, dram, lst in (("th", thT, tht), ("sg", sgT, sgt), ("mk", mkT, mkt)):
            for h in range(2):
                t_ = const.tile([128, RC], F32, tag=f"{name}{h}")
                nc.gpsimd.dma_start(t_[:], dram[h * 128 : (h + 1) * 128, :])
                lst.append(t_)

        lkt = const.tile([128, 1], F32, tag="lkt")
        nc.gpsimd.dma_start(lkt[:], lkb[:])
        selpt = const.tile([128, 2 * RC], F32R, tag="selpt")
        nc.gpsimd.dma_start(selpt[:], selp[:])
        wct = const.tile([RC, 1], F32, tag="wct")
        nc.gpsimd.dma_start(wct[:], wcol[:])

        kap = const.tile([128, 1], F32, tag="kap")
        nc.scalar.activation(kap[:], lkt[:], AF.Exp)
        nkap = const.tile([128, 1], F32, tag="nkap")
        nc.vector.tensor_scalar(nkap[:], kap[:], -1.0, None, OP.mult)

        aa, nb2, mm_, cc_ = [], [], [], []
        for h in range(2):
            tnh = const.tile([128, RC], F32, tag=f"tnh{h}")
            nc.scalar.activation(tnh[:], sgt[h][:], AF.Tanh)
            a_h = const.tile([128, RC], F32, tag=f"a{h}")
            nc.vector.tensor_scalar(a_h[:], tnh[:], kap[:], None, OP.mult)
            na_h = const.tile([128, RC], F32, tag=f"na{h}")
            nc.vector.tensor_scalar(na_h[:], tnh[:], nkap[:], None, OP.mult)
            nb2_h = const.tile([128, RC], F32, tag=f"nb2{h}")
            nc.vector.tensor_mul(nb2_h[:], na_h[:], tht[h][:])
            aa.append(a_h)
            nb2.append(nb2_h)
            m_h = const.tile([128, RC], F32, tag=f"m{h}")
            nc.scalar.activation(m_h[:], mkt[h][:], AF.Sigmoid)
            c_h = const.tile([128, RC], F32, tag=f"c{h}")
            nc.scalar.activation(c_h[:], mkt[h][:], AF.Sigmoid, scale=-1.0)
            mm_.append(m_h)
            cc_.append(c_h)

        lz = psum.tile([RC, B], F32, tag="lz")
        last_ln = None
        for rep in range(reps):
            for blk in range(RC // KBLK):
                gps = []
                sig_insts = []
                for k in range(KBLK):
                    r = blk * KBLK + k
                    s = sp.tile([128, 2 * B], F32, tag="s")
                    for h in range(2):
                        si = nc.scalar.activation(
                            s[:, h * B : (h + 1) * B],
                            xt[h][:],
                            AF.Sigmoid,
                            bias=nb2[h][:, r : r + 1],
                            scale=aa[h][:, r : r + 1],
                        )
                        if last_ln is not None:
                            add_dep_helper(si.ins, last_ln.ins, False,
                                           "act-table phase blocking")
                        sig_insts.append(si)
                    g = gp_.tile([128, 2 * B], F32, tag="g")
                    for h in range(2):
                        nc.vector.tensor_scalar(
                            g[:, h * B : (h + 1) * B],
                            s[:, h * B : (h + 1) * B],
                            mm_[h][:, r : r + 1],
                            cc_[h][:, r : r + 1],
                            OP.mult,
                            OP.add,
                        )
                    gpt = gpp.tile([128, B], F32, tag="gpt")
                    nc.vector.tensor_mul(gpt[:], g[:, 0:B], g[:, B : 2 * B])
                    gps.append(gpt)
                for k in range(KBLK):
                    r = blk * KBLK + k
                    L = lp.tile([128, B], F32R, tag="L")
                    ln_i = nc.scalar.activation(L[:], gps[k][:], AF.Ln)
                    add_dep_helper(ln_i.ins, sig_insts[-1].ins, False,
                                   "act-table phase blocking")
                    last_ln = ln_i
                    lhsp = selpt[:, RC - r : 2 * RC - r]
                    for c in range(B // CH):
                        nc.tensor.matmul(
                            lz[:, c * CH : (c + 1) * CH],
                            lhsp,
                            L[:, c * CH : (c + 1) * CH],
                            start=(r == 0 and rep == 0),
                            stop=(r == RC - 1 and rep == reps - 1),
                        )

        z_sb = const.tile([RC, B], F32, tag="z")
        nc.scalar.activation(z_sb[:], lz[:], AF.Exp)
        yp = psum.tile([1, B], F32, tag="yp")
        for c in range(B // CH):
            nc.tensor.matmul(
                yp[:, c * CH : (c + 1) * CH],
                wct[:],
                z_sb[:, c * CH : (c + 1) * CH],
                start=True,
                stop=True,
            )
        y_sb = const.tile([1, B], F32, tag="ysb")
        nc.vector.tensor_copy(y_sb[:], yp[:])
        nc.sync.dma_start(y[:], y_sb[:])

    nc.compile()
    return nc


def _get_nc(reps=1):
    key = ("nc", reps)
    if key not in _cache:
        _cache[key] = _build(reps)
    return _cache[key]


def _make_in_maps(inputs):
    x = np.ascontiguousarray(inputs["x"], dtype=np.float32)
    th = np.asarray(inputs["th"], dtype=np.float32)
    sg = np.asarray(inputs["sign_param"], dtype=np.float32)
    mk = np.asarray(inputs["mask_logit"], dtype=np.float32)
    lk = float(np.asarray(inputs["log_kappa"], dtype=np.float32).reshape(-1)[0])
    hw = np.asarray(inputs["head_w"], dtype=np.float32)

    xT = np.ascontiguousarray(x.T)
    lkb = np.full((128, 1), lk, dtype=np.float32)
    selp = np.zeros((128, 2 * RC), dtype=np.float32)
    selp[:, RC] = 1.0

    in_maps = []
    for c in range(NCORES):
        sl = slice(c * RC, (c + 1) * RC)
        in_maps.append(
            {
                "xT": xT,
                "thT": np.ascontiguousarray(th[sl].T),
                "sgT": np.ascontiguousarray(sg[sl].T),
                "mkT": np.ascontiguousarray(mk[sl].T),
                "lkb": lkb,
                "wcol": np.ascontiguousarray(hw.reshape(-1)[sl].reshape(RC, 1)),
                "selp": selp,
            }
        )
    return in_maps


def _run_legacy(inputs, reps=1, **spmd_kwargs):
    os.environ.pop("BASS_ACT_ROOT_JSON_PATH", None)
    nc = _get_nc(reps)
    in_maps = _make_in_maps(inputs)
    res = run_bass_kernel_spmd(nc, in_maps, core_ids=list(range(NCORES)),
                               **spmd_kwargs)
    hb = np.asarray(inputs["head_b"], dtype=np.float32).reshape(-1)[0]
    y = np.sum([r["y"][0] for r in res.results], axis=0, dtype=np.float32) + hb
    return y.astype(np.float32), res


def _run(inputs, reps=1, **spmd_kwargs):
    params = _fast_path_params(inputs)
    if params is not None:
        return _run_rk(inputs, params, reps=reps, **spmd_kwargs)
    return _run_legacy(inputs, reps=reps, **spmd_kwargs)


def kernel(**inputs) -> np.ndarray:
    y, _ = _run(inputs)
    return y


# revision 13
# speedup vs baseline: 1.5092x; 1.1596x over previous
"""CornerNet Trainium2 kernel — rank-K separable expansion.

Math (reference):
  t     = kappa * tanh(sign_param) * (x - th)        # (B, R, D)
  s     = sigmoid(t); m = sigmoid(mask_logit)
  gated = 1 - m*(1-s)
  z     = prod_d gated                               # (B, R)
  y     = z @ head_w.T + head_b                      # (B,)

FAST PATH (mask_logit uniform, th == 0 — the actual model):
  ln z[b,r] = sum_d phi(a_rd * x_bd),  phi(u) = ln(1 - m + m*sigmoid(u))
The bivariate kernel phi(a*x) admits a rank-K separable expansion
  phi(a*x) ~= sum_k h_k(sp) * g_k(x)        (SVD of phi on a grid; h_k
                                             absorbs a = kappa*tanh(sp))
so the whole (B,R,D) elementwise pass collapses to K matmuls:
  ln z = sum_k G_k^T-contraction-F_k   with G_k = g_k(x), F_k = h_k(sp).

The arbitrary functions g_k / h_k / exp are evaluated in SINGLE ScalarE
passes by window-packing the `sigmoid_and_others` activation-table set
(selected because it holds sigmoid+tanh+erf+arctan -> ~1030 cubic spline
buckets in one load): each function gets a sub-interval of a table
function's stored side, addressed via the activation instruction's free
input affine (scale*x + bias); the buckets covering that window are refit
to the target function.  Inputs stay strictly on the stored side so
symmetry folding never fires, and all windows live in ONE table set (one
~2.7us ACT_TABLE_LOAD, no switches).

Sharding: data-parallel over batch: core c takes x columns [c*256,(c+1)*256)
and all R=512 rules; no cross-core reduction (host just concatenates).
Layout: D=256 on partitions (two 128-halves side by side on the free axis).
Per core / rep: ACT = K g-passes (128,512) + 1 exp pass (128,1024);
PE = 8K accumulating matmuls (N=256; f32r for k<2, bf16 for k>=2 whose
coefficients are small enough that bf16 noise is negligible) + 4 head
matmuls.  The exp+head tail of rep i is emitted inside rep i+1's k-loop so
the strict-FIFO ACT queue never starves PE at the rep boundary.  PSUM
gotcha: start=True clears has_written for the whole BANK, so exactly one
start/stop per bank per accumulation group.  Defaults K=7 (rel err ~1.8e-3
vs the 2e-2 gate); CN_K / CN_BF16_FROM env override for experiments.
Measured steady-state ~8.6us/rep vs 343us/164us for the per-rule baseline.

FALLBACK (non-uniform mask / th != 0 / missing scipy): the original
per-rule kernel (ScalarE bound) kept verbatim below.
"""

import hashlib
import json
import math
import os
import shutil
import tempfile

import numpy as np
from contextlib import ExitStack

import concourse.bass as bass
import concourse.bacc as bacc
import concourse.mybir as mybir
import concourse.tile as tile
from concourse.bass_utils import run_bass_kernel_spmd
from bass_rust import add_dep_helper

B, D, R = 2048, 256, 512
NCORES = 8
BC = B // NCORES            # 256 batch columns per core
K_RANK = int(os.environ.get("CN_K", "7"))
RC = R // NCORES            # legacy fallback: 64 rules per core
KBLK = 8
CH = 512
F32 = mybir.dt.float32
F32R = mybir.dt.float32r
AF = mybir.ActivationFunctionType
OP = mybir.AluOpType

_cache = {}

TABLE_VERSION = "rk4"

# ======================================================================
# Activation-table window toolkit (sigmoid_and_others set)
# ======================================================================

SET_NAME = "sigmoid_and_others"
# contiguous usable bucket index ranges (inclusive) per table function
BKT_RANGES = {
    "sigmoid": (139, 931),   # x0 -0.5625 .. -99.5625, uniform h=1/8 (desc)
    "tanh": (36, 123),       # x0 0.2656 .. 7.875
    "erf": (951, 1009),      # x0 0.2656 .. 3.66
    "arctan": (1046, 1141),  # x0 0.5156 .. 31.5 (log-spaced)
}
AF_OF = {"sigmoid": AF.Sigmoid, "tanh": AF.Tanh, "erf": AF.Erf,
         "arctan": AF.Arctan}
# uniform-resolution sub-segments usable as windows: (fn, u_lo, u_hi)
SEGMENTS = {
    "tanh_lo":  ("tanh",   0.266, 1.0, 1 / 32, 1 / 32),
    "tanh_mid": ("tanh",   1.0,   4.0, 1 / 16, 1 / 16),
    "erf_lo":   ("erf",    0.266, 1.0, 1 / 32, 1 / 32),
    "erf_mid":  ("erf",    1.0,   3.6, 1 / 16, 1 / 16),
    "atan_hi":  ("arctan", 16.0, 31.9, 1.0, 1.0),
}
SIG_LO, SIG_HI = -99.55, -0.50
SIG_H = 0.125


def _load_set(srcdir):
    d = json.load(open(os.path.join(srcdir, SET_NAME + ".json")))
    bkt = np.fromfile(
        os.path.join(srcdir, SET_NAME + "_bkt.bin"), dtype=np.float32
    ).reshape(d["bkt_entry_cnt"], 8)
    return d, bkt


def _region_table(bkt):
    out = {}
    for fn, (lo, hi) in BKT_RANGES.items():
        x0 = bkt[lo : hi + 1, 4].astype(np.float64)
        dx = np.abs(np.diff(x0))
        # fit over max of adjacent spacings: no extrapolation at octave edges
        w = np.empty_like(x0)
        w[0] = dx[0]
        w[1:-1] = np.maximum(dx[:-1], dx[1:])
        w[-1] = dx[-1]
        out[fn] = (lo, hi, x0, w)
    return out


def _fit_bucket(target, x0, w, npts=33):
    u = np.linspace(x0 - w / 2, x0 + w / 2, npts)
    y = target(u)
    A = np.vander(u - x0, 4, increasing=True)
    coef, *_ = np.linalg.lstsq(A, y, rcond=None)
    return coef


class _Window:
    """g(x) for x in [x_lo, x_hi] == F_fn(scale*x + bias), buckets refit."""

    def __init__(self, fn, u_lo, u_hi, x_lo, x_hi, g):
        self.fn = fn
        s = (u_hi - u_lo) / (x_hi - x_lo)
        b = u_lo - s * x_lo
        self.scale = float(np.float32(s))
        self.bias = float(np.float32(b))
        us = (self.scale * x_lo + self.bias, self.scale * x_hi + self.bias)
        self.u_lo, self.u_hi = min(us), max(us)
        self.g = g

    def apply(self, bkt, regions):
        lo, hi, x0, w = regions[self.fn]
        tgt = lambda u: self.g((np.asarray(u, np.float64) - self.bias) / self.scale)
        n = 0
        for i in range(len(x0)):
            if self.u_lo - 0.45 * w[i] <= x0[i] <= self.u_hi + 0.45 * w[i]:
                bkt[lo + i, 0:4] = _fit_bucket(tgt, x0[i], w[i]).astype(np.float32)
                n += 1
        return n


def _design_windows(mval, kappa, XM, SM, K):
    """SVD of phi(a*x) -> window plans for g_k (x-domain), h_k (sp-domain)
    and the final exp.  Pure numpy + scipy."""
    from scipy.interpolate import CubicSpline

    def phi(u):
        return np.logaddexp(np.log(1.0 - mval), u) - np.logaddexp(0.0, u)

    AMg = kappa * np.tanh(SM)
    ag = np.linspace(-AMg, AMg, 1001)
    xg = np.linspace(-XM, XM, 2001)
    U, S, Vt = np.linalg.svd(phi(np.outer(ag, xg)), full_matrices=False)
    gmax = np.abs(Vt[:K]).max(1)
    gs = [CubicSpline(xg, Vt[k] / gmax[k]) for k in range(K)]
    spg = np.linspace(-SM, SM, 6001)
    fs = [CubicSpline(ag, U[:, k] * S[k] * gmax[k]) for k in range(K)]
    hs = [CubicSpline(spg, fs[k](kappa * np.tanh(spg))) for k in range(K)]

    # sigmoid-band bucket counts per window (validated numerically: K=10
    # end-to-end 6.8e-4, K=12 needs the segment windows below to fit)
    ng = [24, 48, 48, 48, 48, 24, 24, 48, 24, 24, 32, 24]
    nh = [28, 64, 64, 64, 48, 28, 32, 48, 24, 24, 32, 24]
    seg_assign = {("g", 0): "tanh_mid", ("g", 5): "tanh_lo",
                  ("g", 6): "erf_mid", ("g", 8): "erf_lo"}

    cur = SIG_HI
    plans_g, plans_h = {}, {}

    def place_sig(n, dom_lo, dom_hi, g):
        nonlocal cur
        width = n * SIG_H
        u_hi = cur - SIG_H / 2
        u_lo = u_hi - (width - SIG_H)
        cur = cur - width - SIG_H
        assert cur > SIG_LO - SIG_H, "sigmoid band overflow"
        return _Window("sigmoid", u_lo, u_hi, dom_lo, dom_hi, g)

    def place_seg(name, dom_lo, dom_hi, g):
        fn, lo, hi, h_lo, h_hi = SEGMENTS[name]
        return _Window(fn, lo + 0.6 * h_lo, hi - 0.6 * h_hi, dom_lo, dom_hi, g)

    for k in range(K):
        if ("g", k) in seg_assign:
            plans_g[k] = place_seg(seg_assign[("g", k)], -XM, XM, gs[k])
        else:
            plans_g[k] = place_sig(ng[k], -XM, XM, gs[k])
    for k in range(K):
        if ("h", k) in seg_assign:
            plans_h[k] = place_seg(seg_assign[("h", k)], -SM, SM, hs[k])
        else:
            plans_h[k] = place_sig(nh[k], -SM, SM, hs[k])

    # exp on arctan's log-spaced [0.52, 15.9]: u = s*lz + b with s<0 puts the
    # top (important) lz octaves on the finest buckets; covers lz in
    # [-44, -10.8] with rel err <= ~2e-5 at the top
    plan_exp = _Window("arctan", 0.52, 15.9, -10.8, -44.0, np.exp)
    return plans_g, plans_h, plan_exp


def _get_design(mval, kappa, XM, SM, K):
    key = ("design", mval, kappa, XM, SM, K)
    if key not in _cache:
        _cache[key] = _design_windows(mval, kappa, XM, SM, K)
    return _cache[key]


def _gen_act_tables_rk(mval, kappa, XM, SM, K):
    """Build patched act-table dir; returns (act_info_json_path, tag)."""
    from neuronxcc.driver.Job import Job
    from neuronxcc.driver.jobs.support.FindActInfo import findActInfoFile

    src_json = findActInfoFile(Job.getPackageDir(), "gen3")
    srcdir = os.path.dirname(src_json)
    tag = hashlib.md5(
        (TABLE_VERSION + repr((float(mval), float(kappa), float(XM),
                               float(SM), int(K)))).encode()
    ).hexdigest()[:10]
    dstdir = os.path.join(tempfile.gettempdir(), f"cn_rk_{tag}")
    marker = os.path.join(dstdir, "act_info.json")
    if not os.path.isfile(marker):
        plans_g, plans_h, plan_exp = _get_design(mval, kappa, XM, SM, K)
        tmp = dstdir + ".tmp"
        shutil.rmtree(tmp, ignore_errors=True)
        os.makedirs(tmp)
        for f in os.listdir(srcdir):
            shutil.copyfile(os.path.join(srcdir, f), os.path.join(tmp, f))
        d, bkt = _load_set(tmp)
        regions = _region_table(bkt)
        for p in list(plans_g.values()) + list(plans_h.values()) + [plan_exp]:
            p.apply(bkt, regions)
        bkt.tofile(os.path.join(tmp, SET_NAME + "_bkt.bin"))
        shutil.rmtree(dstdir, ignore_errors=True)
        try:
            os.rename(tmp, dstdir)
        except OSError:
            if not os.path.isfile(marker):
                raise
    return marker, tag


# ======================================================================
# Fast kernel: rank-K expansion
# ======================================================================

def _build_rk(reps, tag, plans_g, plans_h, plan_exp, K, bf16_from=None,
              loop_n=0):
    NW = 2 * K + 1
    BF16 = mybir.dt.bfloat16
    def mm_dt(k):
        return BF16 if (bf16_from is not None and k >= bf16_from) else F32R
    nc = bacc.Bacc(None)
    xTc = nc.dram_tensor("xTc", [D, BC], F32, kind="ExternalInput")
    spT = nc.dram_tensor("spT", [D, R], F32, kind="ExternalInput")
    wrow = nc.dram_tensor("wrow", [128, R // 128], F32R, kind="ExternalInput")
    bvec = nc.dram_tensor(f"bv_{tag}", [128, NW], F32, kind="ExternalInput")
    y = nc.dram_tensor("y", [1, BC], F32, kind="ExternalOutput")
    NR = R // 128  # 4 rule chunks

    with tile.TileContext(nc) as tc, ExitStack() as ctx:
        const = ctx.enter_context(tc.tile_pool(name="const", bufs=1))
        gp = ctx.enter_context(tc.tile_pool(name="gp", bufs=4))
        zp = ctx.enter_context(tc.tile_pool(name="zp", bufs=2))
        psum = ctx.enter_context(
            tc.tile_pool(name="psum", bufs=1, space=bass.MemorySpace.PSUM)
        )

        # ---- input staging ----
        xt = const.tile([128, 2 * BC], F32, tag="xt")
        for h in range(2):
            nc.gpsimd.dma_start(xt[:, h * BC : (h + 1) * BC],
                                xTc[h * 128 : (h + 1) * 128, :])
        spt = const.tile([128, 2 * R], F32, tag="spt")
        for h in range(2):
            nc.sync.dma_start(spt[:, h * R : (h + 1) * R],
                              spT[h * 128 : (h + 1) * 128, :])
        wt = const.tile([128, NR], F32R, tag="wt")
        nc.sync.dma_start(wt[:], wrow[:])
        bt = const.tile([128, NW], F32, tag="bt")
        nc.sync.dma_start(bt[:], bvec[:])

        # ---- F_k = h_k(sp), once per execution ----
        fks = []
        for k in range(K):
            w = plans_h[k]
            fk = const.tile([128, 2 * R], mm_dt(k), tag=f"F{k}")
            nc.scalar.activation(fk[:], spt[:], AF_OF[w.fn],
                                 bias=bt[:, K + k : K + k + 1], scale=w.scale)
            fks.append(fk)

        # ---- main loop ----
        lzs = [psum.tile([128, NR * BC], F32, tag=f"lz{p}", name=f"lz{p}")
               for p in range(2)]
        yps = [psum.tile([1, BC], F32, tag=f"yp{p}", name=f"yp{p}")
               for p in range(2)]

        def emit_tail(par):
            # exp + head + copy for the accumulation that finished in lzs[par]
            lz = lzs[par]
            z = zp.tile([128, NR * BC], F32R, tag="z", name="z")
            we = plan_exp
            nc.scalar.activation(z[:], lz[:], AF_OF[we.fn],
                                 bias=bt[:, 2 * K : 2 * K + 1], scale=we.scale)
            yp = yps[par]
            for r in range(NR):
                nc.tensor.matmul(yp[:], wt[:, r : r + 1],
                                 z[:, r * BC : (r + 1) * BC],
                                 start=(r == 0), stop=(r == NR - 1))
            ysb = zp.tile([1, BC], F32, tag="ysb", name="ysb")
            nc.vector.tensor_copy(ysb[:], yp[:])
            return ysb

        def emit_body(par, pending):
            # the k-loop for this rep; the PREVIOUS rep's exp/head tail is
            # emitted mid-loop so ACT never starves PE at the boundary
            lz = lzs[par]
            ysb = None
            for k in range(K):
                w = plans_g[k]
                g = gp.tile([128, 2 * BC], mm_dt(k), tag="G", name="G")
                nc.scalar.activation(g[:], xt[:], AF_OF[w.fn],
                                     bias=bt[:, k : k + 1], scale=w.scale)
                for d in range(2):
                    for r in range(NR):
                        nc.tensor.matmul(
                            lz[:, r * BC : (r + 1) * BC],
                            fks[k][:, d * R + r * 128 : d * R + (r + 1) * 128],
                            g[:, d * BC : (d + 1) * BC],
                            start=(k == 0 and d == 0 and r % 2 == 0),
                            stop=(k == K - 1 and d == 1 and r % 2 == 1),
                        )
                if k == 1 and pending is not None:
                    ysb = emit_tail(pending)
            return ysb

        def emit_reps(n):
            pending = None
            ysb = None
            for rep in range(n):
                t = emit_body(rep % 2, pending)
                ysb = t if t is not None else ysb
                pending = rep % 2
            ysb2 = emit_tail(pending)
            return ysb2

        if loop_n:
            with tc.For_i(0, loop_n, 1):
                ysb = emit_reps(reps)
        else:
            ysb = emit_reps(reps)
        nc.sync.dma_start(y[:], ysb[:])

    nc.compile()

    n_loads = sum(
        1
        for blk in nc.main_func.blocks
        for inst in blk.instructions
        if type(inst).__name__ == "InstLoadActFuncSet"
    )
    if n_loads != 1:
        raise RuntimeError(f"expected 1 act table load, got {n_loads}")
    return nc


def _get_nc_rk(reps, tag, plans, K, loop_n=0):
    bf16_from = _variant_bf16_from()
    key = ("rk", reps, tag, K, bf16_from, loop_n)
    if key not in _cache:
        _cache[key] = _build_rk(reps, tag, *plans, K, bf16_from=bf16_from,
                                loop_n=loop_n)
    return _cache[key]


def _variant_bf16_from():
    v = os.environ.get("CN_BF16_FROM", "2")
    if v in ("", "none"):
        return None
    return int(v)


def _make_in_maps_rk(inputs, tag, plans_g, plans_h, plan_exp, K):
    x = np.ascontiguousarray(np.asarray(inputs["x"], dtype=np.float32))
    sp = np.asarray(inputs["sign_param"], dtype=np.float32)
    hw = np.asarray(inputs["head_w"], dtype=np.float32).reshape(-1)

    xT = np.ascontiguousarray(x.T)                  # (D, B)
    spT = np.ascontiguousarray(sp.T)                # (D, R)
    wrow = np.ascontiguousarray(hw.reshape(R // 128, 128).T)  # (128, R/128)
    biases = ([plans_g[k].bias for k in range(K)]
              + [plans_h[k].bias for k in range(K)] + [plan_exp.bias])
    bvec = np.tile(np.asarray(biases, np.float32), (128, 1))

    in_maps = []
    for c in range(NCORES):
        in_maps.append({
            "xTc": np.ascontiguousarray(xT[:, c * BC : (c + 1) * BC]),
            "spT": spT,
            "wrow": wrow,
            f"bv_{tag}": bvec,
        })
    return in_maps


def _fast_path_params(inputs):
    """Return (m, kappa, XM, SM) if the fast path applies, else None."""
    try:
        import scipy.interpolate  # noqa: F401
    except Exception:
        return None
    th = np.asarray(inputs["th"], dtype=np.float64)
    if th.shape != (R, D) or np.any(th != 0.0):
        return None
    mk = np.asarray(inputs["mask_logit"], dtype=np.float64)
    v = mk.reshape(-1)[0]
    if mk.shape != (R, D) or not np.all(mk == v):
        return None
    x = np.asarray(inputs["x"], dtype=np.float64)
    sp = np.asarray(inputs["sign_param"], dtype=np.float64)
    if x.shape != (B, D) or sp.shape != (R, D):
        return None
    mval = 1.0 / (1.0 + np.exp(-v))
    kappa = math.exp(float(np.asarray(inputs["log_kappa"]).reshape(-1)[0]))
    XM = float(np.float32(np.abs(x).max() * 1.06))
    SM = float(np.float32(np.abs(sp).max() * 1.06))
    if not (0.05 < mval < 0.95) or not (0.05 < kappa < 100.0):
        return None
    if XM > 30.0 or SM > 0.9 or kappa * np.tanh(SM) * XM > 60.0:
        return None
    return float(mval), float(kappa), XM, SM


def _run_rk(inputs, params, reps=1, **spmd_kwargs):
    mval, kappa, XM, SM = params
    K = K_RANK
    json_path, tag = _gen_act_tables_rk(mval, kappa, XM, SM, K)
    os.environ["BASS_ACT_ROOT_JSON_PATH"] = json_path
    plans = _get_design(mval, kappa, XM, SM, K)
    nc = _get_nc_rk(reps, tag, plans, K)
    in_maps = _make_in_maps_rk(inputs, tag, *plans, K)
    res = run_bass_kernel_spmd(nc, in_maps, core_ids=list(range(NCORES)),
                               **spmd_kwargs)
    hb = np.asarray(inputs["head_b"], dtype=np.float32).reshape(-1)[0]
    y = np.concatenate([r["y"][0] for r in res.results]) + hb
    return y.astype(np.float32), res


# ======================================================================
# Legacy fallback kernel (per-rule ScalarE path) — kept verbatim
# ======================================================================

def _build(reps=1):
    nc = bacc.Bacc(None)
    xT = nc.dram_tensor("xT", [D, B], F32, kind="ExternalInput")
    thT = nc.dram_tensor("thT", [D, RC], F32, kind="ExternalInput")
    sgT = nc.dram_tensor("sgT", [D, RC], F32, kind="ExternalInput")
    mkT = nc.dram_tensor("mkT", [D, RC], F32, kind="ExternalInput")
    lkb = nc.dram_tensor("lkb", [128, 1], F32, kind="ExternalInput")
    wcol = nc.dram_tensor("wcol", [RC, 1], F32, kind="ExternalInput")
    selp = nc.dram_tensor("selp", [128, 2 * RC], F32R, kind="ExternalInput")
    y = nc.dram_tensor("y", [1, B], F32, kind="ExternalOutput")

    with tile.TileContext(nc) as tc, ExitStack() as ctx:
        const = ctx.enter_context(tc.tile_pool(name="const", bufs=1))
        sp = ctx.enter_context(tc.tile_pool(name="sp", bufs=2))
        gp_ = ctx.enter_context(tc.tile_pool(name="gp_", bufs=2))
        gpp = ctx.enter_context(tc.tile_pool(name="gpp", bufs=KBLK + 1))
        lp = ctx.enter_context(tc.tile_pool(name="lp", bufs=2))
        psum = ctx.enter_context(
            tc.tile_pool(name="psum", bufs=1, space=bass.MemorySpace.PSUM)
        )

        xt = []
        for h in range(2):
            t_ = const.tile([128, B], F32, tag=f"xt{h}")
            nc.gpsimd.dma_start(t_[:], xT[h * 128 : (h + 1) * 128, :])
            xt.append(t_)

        tht, sgt, mkt = [], [], []
        for name, dram, lst in (("th", thT, tht), ("sg", sgT, sgt), ("mk", mkT, mkt)):
            for h in range(2):
                t_ = const.tile([128, RC], F32, tag=f"{name}{h}")
                nc.gpsimd.dma_start(t_[:], dram[h * 128 : (h + 1) * 128, :])
                lst.append(t_)

        lkt = const.tile([128, 1], F32, tag="lkt")
        nc.gpsimd.dma_start(lkt[:], lkb[:])
        selpt = const.tile([128, 2 * RC], F32R, tag="selpt")
        nc.gpsimd.dma_start(selpt[:], selp[:])
        wct = const.tile([RC, 1], F32, tag="wct")
        nc.gpsimd.dma_start(wct[:], wcol[:])

        kap = const.tile([128, 1], F32, tag="kap")
        nc.scalar.activation(kap[:], lkt[:], AF.Exp)
        nkap = const.tile([128, 1], F32, tag="nkap")
        nc.vector.tensor_scalar(nkap[:], kap[:], -1.0, None, OP.mult)

        aa, nb2, mm_, cc_ = [], [], [], []
        for h in range(2):
            tnh = const.tile([128, RC], F32, tag=f"tnh{h}")
            nc.scalar.activation(tnh[:], sgt[h][:], AF.Tanh)
            a_h = const.tile([128, RC], F32, tag=f"a{h}")
            nc.vector.tensor_scalar(a_h[:], tnh[:], kap[:], None, OP.mult)
            na_h = const.tile([128, RC], F32, tag=f"na{h}")
            nc.vector.tensor_scalar(na_h[:], tnh[:], nkap[:], None, OP.mult)
            nb2_h = const.tile([128, RC], F32, tag=f"nb2{h}")
            nc.vector.tensor_mul(nb2_h[:], na_h[:], tht[h][:])
            aa.append(a_h)
            nb2.append(nb2_h)
            m_h = const.tile([128, RC], F32, tag=f"m{h}")
            nc.scalar.activation(m_h[:], mkt[h][:], AF.Sigmoid)
            c_h = const.tile([128, RC], F32, tag=f"c{h}")
            nc.scalar.activation(c_h[:], mkt[h][:], AF.Sigmoid, scale=-1.0)
            mm_.append(m_h)
            cc_.append(c_h)

        lz = psum.tile([RC, B], F32, tag="lz")
        last_ln = None
        for rep in range(reps):
            for blk in range(RC // KBLK):
                gps = []
                sig_insts = []
                for k in range(KBLK):
                    r = blk * KBLK + k
                    s = sp.tile([128, 2 * B], F32, tag="s")
                    for h in range(2):
                        si = nc.scalar.activation(
                            s[:, h * B : (h + 1) * B],
                            xt[h][:],
                            AF.Sigmoid,
                            bias=nb2[h][:, r : r + 1],
                            scale=aa[h][:, r : r + 1],
                        )
                        if last_ln is not None:
                            add_dep_helper(si.ins, last_ln.ins, False,
                                           "act-table phase blocking")
                        sig_insts.append(si)
                    g = gp_.tile([128, 2 * B], F32, tag="g")
                    for h in range(2):
                        nc.vector.tensor_scalar(
                            g[:, h * B : (h + 1) * B],
                            s[:, h * B : (h + 1) * B],
                            mm_[h][:, r : r + 1],
                            cc_[h][:, r : r + 1],
                            OP.mult,
                            OP.add,
                        )
                    gpt = gpp.tile([128, B], F32, tag="gpt")
                    nc.vector.tensor_mul(gpt[:], g[:, 0:B], g[:, B : 2 * B])
                    gps.append(gpt)
                for k in range(KBLK):
                    r = blk * KBLK + k
                    L = lp.tile([128, B], F32R, tag="L")
                    ln_i = nc.scalar.activation(L[:], gps[k][:], AF.Ln)
                    add_dep_helper(ln_i.ins, sig_insts[-1].ins, False,
                                   "act-table phase blocking")
                    last_ln = ln_i
                    lhsp = selpt[:, RC - r : 2 * RC - r]
                    for c in range(B // CH):
                        nc.tensor.matmul(
                            lz[:, c * CH : (c + 1) * CH],
                            lhsp,
                            L[:, c * CH : (c + 1) * CH],
                            start=(r == 0 and rep == 0),
                            stop=(r == RC - 1 and rep == reps - 1),
                        )

        z_sb = const.tile([RC, B], F32, tag="z")
        nc.scalar.activation(z_sb[:], lz[:], AF.Exp)
        yp = psum.tile([1, B], F32, tag="yp")
        for c in range(B // CH):
            nc.tensor.matmul(
                yp[:, c * CH : (c + 1) * CH],
                wct[:],
                z_sb[:, c * CH : (c + 1) * CH],
                start=True,
                stop=True,
            )
        y_sb = const.tile([1, B], F32, tag="ysb")
        nc.vector.tensor_copy(y_sb[:], yp[:])
        nc.sync.dma_start(y[:], y_sb[:])

    nc.compile()
    return nc


def _get_nc(reps=1):
    key = ("nc", reps)
    if key not in _cache:
        _cache[key] = _build(reps)
    return _cache[key]


def _make_in_maps(inputs):
    x = np.ascontiguousarray(inputs["x"], dtype=np.float32)
    th = np.asarray(inputs["th"], dtype=np.float32)
    sg = np.asarray(inputs["sign_param"], dtype=np.float32)
    mk = np.asarray(inputs["mask_logit"], dtype=np.float32)
    lk = float(np.asarray(inputs["log_kappa"], dtype=np.float32).reshape(-1)[0])
    hw = np.asarray(inputs["head_w"], dtype=np.float32)

    xT = np.ascontiguousarray(x.T)
    lkb = np.full((128, 1), lk, dtype=np.float32)
    selp = np.zeros((128, 2 * RC), dtype=np.float32)
    selp[:, RC] = 1.0

    in_maps = []
    for c in range(NCORES):
        sl = slice(c * RC, (c + 1) * RC)
        in_maps.append(
            {
                "xT": xT,
                "thT": np.ascontiguousarray(th[sl].T),
                "sgT": np.ascontiguousarray(sg[sl].T),
                "mkT": np.ascontiguousarray(mk[sl].T),
                "lkb": lkb,
                "wcol": np.ascontiguousarray(hw.reshape(-1)[sl].reshape(RC, 1)),
                "selp": selp,
            }
        )
    return in_maps


def _run_legacy(inputs, reps=1, **spmd_kwargs):
    os.environ.pop("BASS_ACT_ROOT_JSON_PATH", None)
    nc = _get_nc(reps)
    in_maps = _make_in_maps(inputs)
    res = run_bass_kernel_spmd(nc, in_maps, core_ids=list(range(NCORES)),
                               **spmd_kwargs)
    hb = np.asarray(inputs["head_b"], dtype=np.float32).reshape(-1)[0]
    y = np.sum([r["y"][0] for r in res.results], axis=0, dtype=np.float32) + hb
    return y.astype(np.float32), res


def _run(inputs, reps=1, **spmd_kwargs):
    params = _fast_path_params(inputs)
    if params is not None:
        return _run_rk(inputs, params, reps=reps, **spmd_kwargs)
    return _run_legacy(inputs, reps=reps, **spmd_kwargs)


def kernel(**inputs) -> np.ndarray:
    y, _ = _run(inputs)
    return y
